# revision 10
# baseline (speedup 1.0000x reference)
"""Trainium2 Bass kernel for sliding-window GQA attention block.

Problem (hardcoded):
  B=2, L=2048, DIM=2048, NH=16, NKV=8, HD=128, WIN=128
  out = ( softmax(mask(RoPE(xWq) @ RoPE(xWk)^T * hd^-0.5)) @ (xWv) ) @ Wo^T

Sharding: 8 cores = 2 batches x 4 head-groups (4 q heads + 2 kv heads each).
Each core computes a partial (over its head group) of out[b] in transposed
layout; host sums the 4 partials per batch and transposes back.

All device matmuls run in float32r (full-rate fp32-reduced) with fp32 PSUM
accumulation. Layout choices avoid any on-device transpose:
  - x is passed as xT (DIM, L), weights pre-transposed host-side.
  - q, k are produced as qT/kT [hd, L] (partition = hd).
  - scores are computed transposed: ST[k, q], so softmax sum over k is a
    PE matmul with a ones vector and A@V needs V in natural [l, hd] layout,
    which a second projection pass produces directly.
"""

import sys

sys.path.insert(0, "/opt/trn_rl_repo")

import numpy as np

import concourse.bass as bass
import concourse.mybir as mybir
import concourse.tile as tile
from concourse import bacc
from concourse.bass_utils import run_bass_kernel_spmd

B, L, DIM = 2, 2048, 2048
NH, NKV, HD, WIN = 16, 8, 128, 128
P = 128
NQ_C = 4  # q heads per core
NKV_C = 2  # kv heads per core
KO = DIM // P  # 16 contraction tiles
LB = L // 512  # 4 l-chunks of 512
NPAIR = L // 256  # 8 query-block pairs
SCALE = float(HD) ** -0.5

F32 = mybir.dt.float32
F32R = mybir.dt.float32r


def build_nc(trace_label=""):
    nc = bacc.Bacc(None, target_bir_lowering=False, debug=False)

    xT = nc.dram_tensor("xT", [DIM, L], F32R, kind="ExternalInput")
    wqkvT = nc.dram_tensor("wqkvT", [DIM, 1024], F32R, kind="ExternalInput")
    woT = nc.dram_tensor("woT", [NQ_C * HD, DIM], F32R, kind="ExternalInput")
    cosT = nc.dram_tensor("cosT", [HD, L], F32R, kind="ExternalInput")
    sT = nc.dram_tensor("sT", [HD, L], F32R, kind="ExternalInput")
    maskA = nc.dram_tensor("maskA", [P, 768], F32R, kind="ExternalInput")
    outp = nc.dram_tensor("outp", [DIM, L], F32, kind="ExternalOutput")

    xT_t = xT.rearrange("(ko p) l -> p ko l", p=P)
    wqkvT_t = wqkvT.rearrange("(ko p) m -> p ko m", p=P)
    woT_t = woT.rearrange("(ho p) d -> p ho d", p=P)

    with tile.TileContext(nc) as tc:
        with (
            tc.tile_pool(name="persist", bufs=1) as persist,
            tc.tile_pool(name="consts", bufs=1) as consts,
        ):
            # qkT: douts 0..3 = qT heads, 4..5 = kT kv-heads; [hd, L] each
            qkT = persist.tile([P, 6, L], F32R)
            # vN: natural v [l, (lb), hd of 2 kv heads]
            vN = persist.tile([P, KO, NKV_C * HD], F32R)
            outT = persist.tile([P, NQ_C, L], F32R)  # attn out, [hd, L] per head

            # ---------------- Phase 1: projections ----------------
            # sub-phase 0: q douts (wqkvT cols 0:512)
            # sub-phase 1: k douts (cols 512:768) + v natural (cols 768:1024)
            for sub in range(2):
                with (
                    tc.tile_pool(name=f"w{sub}", bufs=1) as wpool,
                    tc.tile_pool(name=f"xc{sub}", bufs=2) as xpool,
                    tc.tile_pool(name=f"pj{sub}", bufs=4, space="PSUM") as pjp,
                ):
                    w = wpool.tile([P, KO, 512], F32R)
                    nc.sync.dma_start(w[:], wqkvT_t[:, :, sub * 512 : (sub + 1) * 512])
                    for lb in range(LB):
                        xc = xpool.tile([P, KO, 512], F32R, tag="xc")
                        nc.sync.dma_start(
                            xc[:], xT_t[:, :, lb * 512 : (lb + 1) * 512]
                        )
                        nd = 4 if sub == 0 else 2
                        for d in range(nd):
                            ps = pjp.tile([P, 512], F32, tag="pj")
                            for k in range(KO):
                                nc.tensor.matmul(
                                    ps[:],
                                    w[:, k, d * P : (d + 1) * P],
                                    xc[:, k, :],
                                    start=(k == 0),
                                    stop=(k == KO - 1),
                                )
                            dd = d if sub == 0 else 4 + d
                            nc.scalar.activation(
                                qkT[:, dd, lb * 512 : (lb + 1) * 512],
                                ps[:],
                                mybir.ActivationFunctionType.Copy,
                            )
                        if sub == 1:
                            # v natural: out[l, hd] ; lhsT = xT chunk, rhs = wv
                            for j in range(4):
                                ps = pjp.tile([P, 256], F32, tag="pjv")
                                for k in range(KO):
                                    nc.tensor.matmul(
                                        ps[:],
                                        xc[:, k, j * P : (j + 1) * P],
                                        w[:, k, 256:512],
                                        start=(k == 0),
                                        stop=(k == KO - 1),
                                    )
                                nc.scalar.activation(
                                    vN[:, lb * 4 + j, :],
                                    ps[:],
                                    mybir.ActivationFunctionType.Copy,
                                )

            # ---------------- Phase 2: RoPE on q (4) + k (2) ----------------
            with (
                tc.tile_pool(name="trig", bufs=1) as trig,
                tc.tile_pool(name="rtmp", bufs=4) as rtmp,
            ):
                cos_sb = trig.tile([P, L], F32R)
                s_sb = trig.tile([P, L], F32R)
                nc.sync.dma_start(cos_sb[:], cosT[:, :])
                nc.sync.dma_start(s_sb[:], sT[:, :])
                H = HD // 2
                for d in range(6):
                    base = qkT[:, d, :]
                    sw = rtmp.tile([P, L], F32R, tag="sw")
                    u = rtmp.tile([P, L], F32R, tag="u")
                    nc.gpsimd.tensor_copy(sw[0:H, :], base[H:P, :])
                    nc.gpsimd.tensor_copy(sw[H:P, :], base[0:H, :])
                    nc.vector.tensor_mul(u[:], base, cos_sb[:])
                    nc.vector.tensor_mul(sw[:], sw[:], s_sb[:])
                    nc.vector.tensor_add(base, u[:], sw[:])

            # ---------------- Phase 3: attention ----------------
            with (
                tc.tile_pool(name="mask", bufs=1) as maskp,
                tc.tile_pool(name="apool", bufs=4) as apool,
                tc.tile_pool(name="zpool", bufs=2) as zpool,
                tc.tile_pool(name="st_ps", bufs=2, space="PSUM") as stp,
                tc.tile_pool(name="z_ps", bufs=1, space="PSUM") as zpp,
                tc.tile_pool(name="o_ps", bufs=1, space="PSUM") as opp,
            ):
                mask_sb = maskp.tile([P, 768], F32R)
                nc.sync.dma_start(mask_sb[:], maskA[:, :])

                for p in range(NPAIR):
                    kbs = [0, 1] if p == 0 else [2 * p - 1, 2 * p, 2 * p + 1]
                    r0 = 1 if p == 0 else 0  # first active region index
                    lo = r0 * 256
                    z_ps = zpp.tile([1, 1024], F32, tag="z")
                    o_ps = opp.tile([P, 1024], F32, tag="o")
                    a_tiles = []
                    for h in range(NQ_C):
                        kvh = h // 2
                        st = stp.tile([P, 768], F32, tag="st")
                        for i, kb in enumerate(kbs):
                            reg = r0 + i
                            nc.tensor.matmul(
                                st[:, reg * 256 : (reg + 1) * 256],
                                qkT[:, 4 + kvh, kb * P : (kb + 1) * P],
                                qkT[:, h, p * 256 : (p + 1) * 256],
                                start=True,
                                stop=True,
                            )
                        a = apool.tile([P, 768], F32R, tag="a")
                        nc.scalar.activation(
                            a[:, lo:768],
                            st[:, lo:768],
                            mybir.ActivationFunctionType.Exp,
                            scale=SCALE,
                        )
                        nc.vector.tensor_mul(
                            a[:, lo:768], a[:, lo:768], mask_sb[:, lo:768]
                        )
                        a_tiles.append(a)
                        for i, kb in enumerate(kbs):
                            reg = r0 + i
                            nc.tensor.matmul(
                                z_ps[0:1, h * 256 : (h + 1) * 256],
                                mask_sb[:, 383:384],  # all-ones column of Mb
                                a[:, reg * 256 : (reg + 1) * 256],
                                start=(i == 0),
                                stop=(i == len(kbs) - 1),
                            )
                        for i, kb in enumerate(kbs):
                            reg = r0 + i
                            nc.tensor.matmul(
                                o_ps[:, h * 256 : (h + 1) * 256],
                                vN[:, kb, kvh * HD : (kvh + 1) * HD],
                                a[:, reg * 256 : (reg + 1) * 256],
                                start=(i == 0),
                                stop=(i == len(kbs) - 1),
                            )
                    # normalize: recip of Z, broadcast to 128 partitions, mult
                    recip = zpool.tile([1, 1024], F32, tag="recip")
                    zb = zpool.tile([P, 1024], F32, tag="zb")
                    nc.vector.reciprocal(recip[:], z_ps[0:1, :])
                    nc.sync.dma_start(zb[0:1, :], recip[0:1, :])
                    pp = 1
                    while pp < P:
                        nc.sync.dma_start(zb[pp : 2 * pp, :], zb[0:pp, :])
                        pp *= 2
                    for h in range(NQ_C):
                        nc.vector.tensor_mul(
                            outT[:, h, p * 256 : (p + 1) * 256],
                            o_ps[:, h * 256 : (h + 1) * 256],
                            zb[:, h * 256 : (h + 1) * 256],
                        )

            # ---------------- Phase 4: output projection ----------------
            with (
                tc.tile_pool(name="wo", bufs=1) as wop,
                tc.tile_pool(name="oev", bufs=4) as oev,
                tc.tile_pool(name="op_ps", bufs=4, space="PSUM") as opps,
            ):
                wo_sb = wop.tile([P, NQ_C, DIM], F32R)
                nc.sync.dma_start(wo_sb[:], woT_t[:, :, :])
                for lb in range(LB):
                    for d in range(DIM // P):
                        ps = opps.tile([P, 512], F32, tag="op")
                        for h in range(NQ_C):
                            nc.tensor.matmul(
                                ps[:],
                                wo_sb[:, h, d * P : (d + 1) * P],
                                outT[:, h, lb * 512 : (lb + 1) * 512],
                                start=(h == 0),
                                stop=(h == NQ_C - 1),
                            )
                        ot = oev.tile([P, 512], F32, tag="oe")
                        nc.scalar.activation(
                            ot[:], ps[:], mybir.ActivationFunctionType.Copy
                        )
                        nc.sync.dma_start(
                            outp[d * P : (d + 1) * P, lb * 512 : (lb + 1) * 512],
                            ot[:],
                        )

    nc.compile()
    return nc


_NC_CACHE = {}


def _get_nc():
    if "nc" not in _NC_CACHE:
        _NC_CACHE["nc"] = build_nc()
    return _NC_CACHE["nc"]


def _build_masks():
    k = np.arange(P)[:, None]  # partition = key pos within block
    r = np.arange(P)[None, :]  # free = query pos within block
    ut = (r <= k).astype(np.float32)  # delta = -1
    lt = (r >= k).astype(np.float32)  # delta = 0
    z = np.zeros((P, P), np.float32)
    ma = np.concatenate([ut, z], axis=1)
    mb = np.concatenate([lt, ut], axis=1)
    mc = np.concatenate([z, lt], axis=1)
    return np.ascontiguousarray(np.concatenate([ma, mb, mc], axis=1))


def _shard_inputs(**inputs):
    x = np.asarray(inputs["x"], np.float32)
    cos = np.asarray(inputs["cos"], np.float32)
    sin = np.asarray(inputs["sin"], np.float32)
    wq = np.asarray(inputs["wq"], np.float32)
    wk = np.asarray(inputs["wk"], np.float32)
    wv = np.asarray(inputs["wv"], np.float32)
    wo = np.asarray(inputs["wo"], np.float32)

    cosT = np.ascontiguousarray(cos.T)
    sT = np.ascontiguousarray(sin.T).copy()
    sT[: HD // 2] = -sT[: HD // 2]
    maskA = _build_masks()

    in_maps = []
    for c in range(8):
        b, g = c // 4, c % 4
        xT = np.ascontiguousarray(x[b].T)
        wq_g = wq[g * 512 : (g + 1) * 512]  # 4 q heads
        wk_g = wk[g * 256 : (g + 1) * 256]  # 2 kv heads
        wv_g = wv[g * 256 : (g + 1) * 256]
        wqkvT = np.ascontiguousarray(
            np.concatenate([wq_g, wk_g, wv_g], axis=0).T
        )
        woT = np.ascontiguousarray(wo[:, g * 512 : (g + 1) * 512].T)
        in_maps.append(
            {
                "xT": xT,
                "wqkvT": wqkvT,
                "woT": woT,
                "cosT": cosT,
                "sT": sT,
                "maskA": maskA,
            }
        )
    return in_maps


def kernel(**inputs):
    in_maps = _shard_inputs(**inputs)
    nc = _get_nc()
    res = run_bass_kernel_spmd(nc, in_maps, list(range(8)))
    outs = [r["outp"] for r in res.results]
    out = np.empty((B, L, DIM), np.float32)
    for b in range(B):
        acc = outs[b * 4].astype(np.float64)
        for g in range(1, 4):
            acc += outs[b * 4 + g]
        out[b] = acc.T.astype(np.float32)
    return out


# revision 14
# speedup vs baseline: 1.1208x; 1.1208x over previous
"""Trainium2 Bass kernel for sliding-window GQA attention block.

Problem (hardcoded):
  B=2, L=2048, DIM=2048, NH=16, NKV=8, HD=128, WIN=128
  out = ( softmax(mask(RoPE(xWq) @ RoPE(xWk)^T * hd^-0.5)) @ (xWv) ) @ Wo^T

Sharding: 8 cores = 2 batches x 4 head-groups (4 q heads + 2 kv heads each).
Each core computes a partial (over its head group) of out[b] in transposed
layout; host sums the 4 partials per batch and transposes back.

All device matmuls run in float32r (full-rate fp32-reduced) with fp32 PSUM
accumulation. Layout choices avoid any on-device transpose:
  - x is passed as xT (DIM, L), weights pre-transposed host-side.
  - q, k are produced as qT/kT [hd, L] (partition = hd).
  - scores are computed transposed: ST[k, q]; the softmax sum over k is a
    PE matmul with a ones column, packed into region 3 of the score PSUM
    tile; 1/Z is broadcast across partitions with a K=1 PE outer product
    into region 0 after the exp has consumed it.
  - V is produced in natural [l, hd] layout by a second projection pass, so
    A@V contracts over k on partitions directly.

Scheduling: RoPE of k overlaps the q-projection; RoPE of q_h overlaps
attention of head h-1; out-projection streams wo in 128-column chunks.
"""

import sys

sys.path.insert(0, "/opt/trn_rl_repo")

import numpy as np

import concourse.bass as bass
import concourse.mybir as mybir
import concourse.tile as tile
from concourse import bacc
from concourse.bass_utils import run_bass_kernel_spmd

B, L, DIM = 2, 2048, 2048
NH, NKV, HD, WIN = 16, 8, 128, 128
P = 128
NQ_C = 4  # q heads per core
NKV_C = 2  # kv heads per core
KO = DIM // P  # 16 contraction tiles
LB = L // 512  # 4 l-chunks of 512
NPAIR = L // 256  # 8 query-block pairs
SCALE = float(HD) ** -0.5

F32 = mybir.dt.float32
F32R = mybir.dt.float32r

Copy = mybir.ActivationFunctionType.Copy
Exp = mybir.ActivationFunctionType.Exp

ONES_COL = slice(383, 384)  # all-ones column of Mb (LT col 127)
ONES_ROW = slice(256, 384)  # all-ones row 0 of Mb's LT half


def build_nc():
    nc = bacc.Bacc(None, target_bir_lowering=False, debug=False)

    xT = nc.dram_tensor("xT", [DIM, L], F32R, kind="ExternalInput")
    wqkvT = nc.dram_tensor("wqkvT", [DIM, 1024], F32R, kind="ExternalInput")
    woT = nc.dram_tensor("woT", [NQ_C * HD, DIM], F32R, kind="ExternalInput")
    cosT = nc.dram_tensor("cosT", [HD, L], F32R, kind="ExternalInput")
    sT = nc.dram_tensor("sT", [HD, L], F32R, kind="ExternalInput")
    maskA = nc.dram_tensor("maskA", [P, 768], F32R, kind="ExternalInput")
    outp = nc.dram_tensor("outp", [DIM, L], F32, kind="ExternalOutput")

    xT_t = xT.rearrange("(ko p) l -> p ko l", p=P)
    wqkvT_t = wqkvT.rearrange("(ko p) m -> p ko m", p=P)
    woT_t = woT.rearrange("(ho p) d -> p ho d", p=P)

    with tile.TileContext(nc) as tc:
        with tc.tile_pool(name="persist", bufs=1) as persist:
            # qkT: douts 0..3 = qT heads, 4..5 = kT kv-heads; [hd, L] each
            qkT = persist.tile([P, 6, L], F32R)
            # vN: natural v [l(128), lblock(16), hd of 2 kv heads(256)]
            vN = persist.tile([P, KO, NKV_C * HD], F32R)

            # ---- Phase 1a: k projection (transposed) + v (natural) ----
            with (
                tc.tile_pool(name="wkv", bufs=1) as wpool,
                tc.tile_pool(name="xckv", bufs=2) as xpool,
                tc.tile_pool(name="pjkv", bufs=4, space="PSUM") as pjp,
            ):
                w = wpool.tile([P, KO, 512], F32R)
                nc.sync.dma_start(w[:], wqkvT_t[:, :, 512:1024])
                for lb in range(LB):
                    xc = xpool.tile([P, KO, 512], F32R, tag="xc")
                    nc.sync.dma_start(xc[:], xT_t[:, :, lb * 512 : (lb + 1) * 512])
                    for d in range(2):  # kT for 2 kv heads
                        ps = pjp.tile([P, 512], F32, tag="pj")
                        for k in range(KO):
                            nc.tensor.matmul(
                                ps[:],
                                w[:, k, d * P : (d + 1) * P],
                                xc[:, k, :],
                                start=(k == 0),
                                stop=(k == KO - 1),
                            )
                        nc.scalar.activation(
                            qkT[:, 4 + d, lb * 512 : (lb + 1) * 512], ps[:], Copy
                        )
                    for j in range(4):  # v natural: lhsT = xT chunk
                        ps = pjp.tile([P, 256], F32, tag="pjv")
                        for k in range(KO):
                            nc.tensor.matmul(
                                ps[:],
                                xc[:, k, j * P : (j + 1) * P],
                                w[:, k, 256:512],
                                start=(k == 0),
                                stop=(k == KO - 1),
                            )
                        nc.scalar.activation(vN[:, lb * 4 + j, :], ps[:], Copy)

            with (
                tc.tile_pool(name="trig", bufs=1) as trig,
                tc.tile_pool(name="rtmp", bufs=2) as rtmp,
            ):
                cos_sb = trig.tile([P, L], F32R)
                s_sb = trig.tile([P, L], F32R)
                nc.sync.dma_start(cos_sb[:], cosT[:, :])
                nc.sync.dma_start(s_sb[:], sT[:, :])
                H = HD // 2

                def rope(d):
                    # in-place: base = base*cos + swap(base)*sT
                    base = qkT[:, d, :]
                    sw = rtmp.tile([P, L], F32R, tag="sw")
                    nc.gpsimd.tensor_copy(sw[0:H, :], base[H:P, :])
                    nc.gpsimd.tensor_copy(sw[H:P, :], base[0:H, :])
                    nc.vector.tensor_mul(base, base, cos_sb[:])
                    nc.vector.tensor_mul(sw[:], sw[:], s_sb[:])
                    nc.vector.tensor_add(base, base, sw[:])

                # RoPE on k overlaps the q-projection below (independent)
                rope(4)
                rope(5)

                # ---- Phase 1b: q projection ----
                with (
                    tc.tile_pool(name="wq", bufs=1) as wpool,
                    tc.tile_pool(name="xcq", bufs=2) as xpool,
                    tc.tile_pool(name="pjq", bufs=4, space="PSUM") as pjp,
                ):
                    w = wpool.tile([P, KO, 512], F32R)
                    nc.sync.dma_start(w[:], wqkvT_t[:, :, 0:512])
                    for lb in range(LB):
                        xc = xpool.tile([P, KO, 512], F32R, tag="xc")
                        nc.sync.dma_start(
                            xc[:], xT_t[:, :, lb * 512 : (lb + 1) * 512]
                        )
                        for d in range(4):
                            ps = pjp.tile([P, 512], F32, tag="pj")
                            for k in range(KO):
                                nc.tensor.matmul(
                                    ps[:],
                                    w[:, k, d * P : (d + 1) * P],
                                    xc[:, k, :],
                                    start=(k == 0),
                                    stop=(k == KO - 1),
                                )
                            nc.scalar.activation(
                                qkT[:, d, lb * 512 : (lb + 1) * 512], ps[:], Copy
                            )

                # ---- attention (head-outer, RoPE-q interleaved) ----
                with (
                    tc.tile_pool(name="outTp", bufs=1) as outTp,
                    tc.tile_pool(name="mask", bufs=1) as maskp,
                    tc.tile_pool(name="apool", bufs=4) as apool,
                    tc.tile_pool(name="rpool", bufs=2) as rpool,
                    tc.tile_pool(name="st_ps", bufs=2, space="PSUM") as stp,
                    tc.tile_pool(name="oop_ps", bufs=4, space="PSUM") as opp,
                ):
                    outT = outTp.tile([P, NQ_C, L], F32R)
                    mask_sb = maskp.tile([P, 768], F32R)
                    nc.sync.dma_start(mask_sb[:], maskA[:, :])

                    def attn_head(h):
                        kvh = h // 2
                        for p in range(NPAIR):
                            kbs = (
                                [0, 1] if p == 0 else [2 * p - 1, 2 * p, 2 * p + 1]
                            )
                            r0 = 1 if p == 0 else 0
                            lo = r0 * 256
                            qs = slice(p * 256, (p + 1) * 256)
                            st = stp.tile([P, 1024], F32, tag="st")
                            for i, kb in enumerate(kbs):
                                reg = r0 + i
                                nc.tensor.matmul(
                                    st[:, reg * 256 : (reg + 1) * 256],
                                    qkT[:, 4 + kvh, kb * P : (kb + 1) * P],
                                    qkT[:, h, qs],
                                    start=True,
                                    stop=True,
                                )
                            a = apool.tile([P, 768], F32R, tag="a")
                            nc.scalar.activation(
                                a[:, lo:768], st[:, lo:768], Exp, scale=SCALE
                            )
                            nc.gpsimd.tensor_mul(
                                a[:, lo:768], a[:, lo:768], mask_sb[:, lo:768]
                            )
                            for i, kb in enumerate(kbs):  # Z into st region 3
                                reg = r0 + i
                                nc.tensor.matmul(
                                    st[0:1, 768:1024],
                                    mask_sb[:, ONES_COL],
                                    a[:, reg * 256 : (reg + 1) * 256],
                                    start=(i == 0),
                                    stop=(i == len(kbs) - 1),
                                )
                            recip = rpool.tile([1, 256], F32R, tag="recip")
                            with nc.allow_low_precision(
                                reason="softmax denom recip; well-conditioned"
                            ):
                                nc.vector.reciprocal(recip[:], st[0:1, 768:1024])
                            # broadcast 1/Z across partitions into region 0,
                            # then stage to SBUF (DVE can read only one PSUM
                            # operand)
                            nc.tensor.matmul(
                                st[:, 0:256],
                                mask_sb[0:1, ONES_ROW],
                                recip[0:1, :],
                                start=True,
                                stop=True,
                            )
                            zb_sb = rpool.tile([P, 256], F32, tag="zb")
                            nc.scalar.activation(zb_sb[:], st[:, 0:256], Copy)
                            o_ps = opp.tile([P, 512], F32, tag="oop")
                            for i, kb in enumerate(kbs):
                                reg = r0 + i
                                nc.tensor.matmul(
                                    o_ps[:, 0:256],
                                    vN[:, kb, kvh * HD : (kvh + 1) * HD],
                                    a[:, reg * 256 : (reg + 1) * 256],
                                    start=(i == 0),
                                    stop=(i == len(kbs) - 1),
                                )
                            nc.vector.tensor_mul(
                                outT[:, h, qs], o_ps[:, 0:256], zb_sb[:]
                            )

                    rope(0)
                    rope(1)
                    attn_head(0)
                    rope(2)
                    attn_head(1)
                    rope(3)
                    attn_head(2)
                    attn_head(3)

                    # ---- out-projection, wo streamed in 128-col chunks ----
                    with (
                        tc.tile_pool(name="wo", bufs=3) as wop,
                        tc.tile_pool(name="oev", bufs=4) as oev,
                    ):
                        for d in range(DIM // P):
                            woc = wop.tile([P, NQ_C, P], F32R, tag="woc")
                            nc.sync.dma_start(
                                woc[:], woT_t[:, :, d * P : (d + 1) * P]
                            )
                            for lb in range(LB):
                                ps = opp.tile([P, 512], F32, tag="oop")
                                for h in range(NQ_C):
                                    nc.tensor.matmul(
                                        ps[:],
                                        woc[:, h, :],
                                        outT[:, h, lb * 512 : (lb + 1) * 512],
                                        start=(h == 0),
                                        stop=(h == NQ_C - 1),
                                    )
                                ot = oev.tile([P, 512], F32, tag="oe")
                                nc.scalar.activation(ot[:], ps[:], Copy)
                                nc.sync.dma_start(
                                    outp[
                                        d * P : (d + 1) * P,
                                        lb * 512 : (lb + 1) * 512,
                                    ],
                                    ot[:],
                                )

    nc.compile()
    return nc


_NC_CACHE = {}


def _get_nc():
    if "nc" not in _NC_CACHE:
        _NC_CACHE["nc"] = build_nc()
    return _NC_CACHE["nc"]


def _build_masks():
    k = np.arange(P)[:, None]  # partition = key pos within block
    r = np.arange(P)[None, :]  # free = query pos within block
    ut = (r <= k).astype(np.float32)  # delta = -1
    lt = (r >= k).astype(np.float32)  # delta = 0
    z = np.zeros((P, P), np.float32)
    ma = np.concatenate([ut, z], axis=1)
    mb = np.concatenate([lt, ut], axis=1)
    mc = np.concatenate([z, lt], axis=1)
    return np.ascontiguousarray(np.concatenate([ma, mb, mc], axis=1))


def _shard_inputs(**inputs):
    x = np.asarray(inputs["x"], np.float32)
    cos = np.asarray(inputs["cos"], np.float32)
    sin = np.asarray(inputs["sin"], np.float32)
    wq = np.asarray(inputs["wq"], np.float32)
    wk = np.asarray(inputs["wk"], np.float32)
    wv = np.asarray(inputs["wv"], np.float32)
    wo = np.asarray(inputs["wo"], np.float32)

    cosT = np.ascontiguousarray(cos.T)
    sT = np.ascontiguousarray(sin.T).copy()
    sT[: HD // 2] = -sT[: HD // 2]
    maskA = _build_masks()

    in_maps = []
    for c in range(8):
        b, g = c // 4, c % 4
        xT = np.ascontiguousarray(x[b].T)
        wq_g = wq[g * 512 : (g + 1) * 512]  # 4 q heads
        wk_g = wk[g * 256 : (g + 1) * 256]  # 2 kv heads
        wv_g = wv[g * 256 : (g + 1) * 256]
        wqkvT = np.ascontiguousarray(
            np.concatenate([wq_g, wk_g, wv_g], axis=0).T
        )
        woT = np.ascontiguousarray(wo[:, g * 512 : (g + 1) * 512].T)
        in_maps.append(
            {
                "xT": xT,
                "wqkvT": wqkvT,
                "woT": woT,
                "cosT": cosT,
                "sT": sT,
                "maskA": maskA,
            }
        )
    return in_maps


def kernel(**inputs):
    in_maps = _shard_inputs(**inputs)
    nc = _get_nc()
    res = run_bass_kernel_spmd(nc, in_maps, list(range(8)))
    outs = [r["outp"] for r in res.results]
    out = np.empty((B, L, DIM), np.float32)
    for b in range(B):
        acc = outs[b * 4].astype(np.float64)
        for g in range(1, 4):
            acc += outs[b * 4 + g]
        out[b] = acc.T.astype(np.float32)
    return out


# revision 15
# speedup vs baseline: 1.3797x; 1.2309x over previous
"""Trainium2 Bass kernel for sliding-window GQA attention block.

Problem (hardcoded):
  B=2, L=2048, DIM=2048, NH=16, NKV=8, HD=128, WIN=128
  out = ( softmax(mask(RoPE(xWq) @ RoPE(xWk)^T * hd^-0.5)) @ (xWv) ) @ Wo^T

Sharding: 8 cores = 2 batches x 4 head-groups (4 q heads + 2 kv heads each).
Each core computes a partial (over its head group) of out[b] in transposed
layout; host sums the 4 partials per batch and transposes back.

All device matmuls run in float32r (full-rate fp32-reduced) with fp32 PSUM
accumulation. No on-device transposes:
  - x is passed as xT (DIM, L), weights pre-transposed host-side.
  - q, k are produced as qT/kT [hd, L]; V in natural [l, hd] layout via a
    second projection pass with xT as the stationary operand.
  - scores are computed transposed ST[k, q]; softmax sum over k is a PE
    matmul with a ones column into region 3 of the score PSUM tile; 1/Z is
    partition-broadcast with a K=1 PE outer product into region 0.

Scheduling: weight/x DMAs split per k-tile so matmuls ramp immediately;
RoPE(k) overlaps q-projection; RoPE(q_h) overlaps attention of earlier
heads; x chunks are reused across the two projection passes (reverse lb
order); out-projection streams wo in 128-column chunks.
"""

import sys

sys.path.insert(0, "/opt/trn_rl_repo")

import numpy as np

import concourse.bass as bass
import concourse.mybir as mybir
import concourse.tile as tile
from concourse import bacc
from concourse.bass_utils import run_bass_kernel_spmd

B, L, DIM = 2, 2048, 2048
NH, NKV, HD, WIN = 16, 8, 128, 128
P = 128
NQ_C = 4  # q heads per core
NKV_C = 2  # kv heads per core
KO = DIM // P  # 16 contraction tiles
LB = L // 512  # 4 l-chunks of 512
NPAIR = L // 256  # 8 query-block pairs
SCALE = float(HD) ** -0.5

F32 = mybir.dt.float32
F32R = mybir.dt.float32r

Copy = mybir.ActivationFunctionType.Copy
Exp = mybir.ActivationFunctionType.Exp

ONES_COL = slice(383, 384)  # all-ones column of Mb (LT col 127)


def build_nc():
    nc = bacc.Bacc(None, target_bir_lowering=False, debug=False)

    xT = nc.dram_tensor("xT", [DIM, L], F32R, kind="ExternalInput")
    wqkvT = nc.dram_tensor("wqkvT", [DIM, 1024], F32R, kind="ExternalInput")
    woT = nc.dram_tensor("woT", [NQ_C * HD, DIM], F32R, kind="ExternalInput")
    cosT = nc.dram_tensor("cosT", [HD, L], F32R, kind="ExternalInput")
    sT = nc.dram_tensor("sT", [HD, L], F32R, kind="ExternalInput")
    maskA = nc.dram_tensor("maskA", [P, 768], F32R, kind="ExternalInput")
    outp = nc.dram_tensor("outp", [DIM, L], F32, kind="ExternalOutput")

    xT_t = xT.rearrange("(ko p) l -> p ko l", p=P)
    wqkvT_t = wqkvT.rearrange("(ko p) m -> p ko m", p=P)
    woT_t = woT.rearrange("(ho p) d -> p ho d", p=P)

    with (
        tile.TileContext(nc) as tc,
        tc.tile_pool(name="persist", bufs=1) as persist,
        tc.tile_pool(name="trig", bufs=1) as trig,
        tc.tile_pool(name="rtmp", bufs=2) as rtmp,
    ):
        # qkT: douts 0..3 = qT heads, 4..5 = kT kv-heads; [hd, L] each
        qkT = persist.tile([P, 6, L], F32R)
        # vN: natural v [l(128), lblock(16), hd of 2 kv heads(256)]
        vN = persist.tile([P, KO, NKV_C * HD], F32R)

        cos_sb = trig.tile([P, L], F32R)
        s_sb = trig.tile([P, L], F32R)
        nc.sync.dma_start(cos_sb[:], cosT[:, :])
        nc.sync.dma_start(s_sb[:], sT[:, :])
        H = HD // 2

        def rope(d):
            # in-place: base = base*cos + swap(base)*sT ; swap via DMA
            base = qkT[:, d, :]
            sw = rtmp.tile([P, L], F32R, tag="sw")
            nc.sync.dma_start(sw[0:H, :], base[H:P, :])
            nc.sync.dma_start(sw[H:P, :], base[0:H, :])
            nc.vector.tensor_mul(base, base, cos_sb[:])
            nc.vector.tensor_mul(sw[:], sw[:], s_sb[:])
            nc.vector.tensor_add(base, base, sw[:])

        with tc.tile_pool(name="xcp", bufs=2) as xpool:
            xcs = {}

            def load_xc(lb, split):
                xc = xpool.tile([P, KO, 512], F32R, tag="xc")
                src = xT_t[:, :, lb * 512 : (lb + 1) * 512]
                if split:
                    for k in range(KO):
                        nc.sync.dma_start(xc[:, k, :], src[:, k, :])
                else:
                    nc.sync.dma_start(xc[:], src)
                xcs[lb] = xc
                return xc

            # ---- Phase 1a: k projection (transposed) + v (natural) ----
            with (
                tc.tile_pool(name="wkv", bufs=1) as wpool,
                tc.tile_pool(name="pjkv", bufs=4, space="PSUM") as pjp,
            ):
                w = wpool.tile([P, KO, 512], F32R)
                for k in range(KO):
                    nc.sync.dma_start(w[:, k, :], wqkvT_t[:, k, 512:1024])
                for lb in range(LB):
                    xc = load_xc(lb, split=(lb == 0))
                    for d in range(2):  # kT for 2 kv heads
                        ps = pjp.tile([P, 512], F32, tag="pj")
                        for k in range(KO):
                            nc.tensor.matmul(
                                ps[:],
                                w[:, k, d * P : (d + 1) * P],
                                xc[:, k, :],
                                start=(k == 0),
                                stop=(k == KO - 1),
                            )
                        nc.scalar.activation(
                            qkT[:, 4 + d, lb * 512 : (lb + 1) * 512], ps[:], Copy
                        )
                    for j in range(4):  # v natural: lhsT = xT chunk
                        ps = pjp.tile([P, 256], F32, tag="pjv")
                        for k in range(KO):
                            nc.tensor.matmul(
                                ps[:],
                                xc[:, k, j * P : (j + 1) * P],
                                w[:, k, 256:512],
                                start=(k == 0),
                                stop=(k == KO - 1),
                            )
                        nc.scalar.activation(vN[:, lb * 4 + j, :], ps[:], Copy)

            # RoPE on k overlaps the q-projection below (independent)
            rope(4)
            rope(5)

            # ---- Phase 1b: q projection (reuses resident x chunks) ----
            with (
                tc.tile_pool(name="wq", bufs=1) as wpool,
                tc.tile_pool(name="pjq", bufs=4, space="PSUM") as pjp,
            ):
                w = wpool.tile([P, KO, 512], F32R)
                for k in range(KO):
                    nc.sync.dma_start(w[:, k, :], wqkvT_t[:, k, 0:512])
                for lb in (3, 2, 1, 0):
                    xc = xcs[lb] if lb >= 2 else load_xc(lb, split=False)
                    for d in range(4):
                        ps = pjp.tile([P, 512], F32, tag="pj")
                        for k in range(KO):
                            nc.tensor.matmul(
                                ps[:],
                                w[:, k, d * P : (d + 1) * P],
                                xc[:, k, :],
                                start=(k == 0),
                                stop=(k == KO - 1),
                            )
                        nc.scalar.activation(
                            qkT[:, d, lb * 512 : (lb + 1) * 512], ps[:], Copy
                        )

        # ---- attention (head-outer, RoPE-q interleaved) + out-proj ----
        with (
            tc.tile_pool(name="outTp", bufs=1) as outTp,
            tc.tile_pool(name="consts", bufs=1) as constsp,
            tc.tile_pool(name="apool", bufs=4) as apool,
            tc.tile_pool(name="rpool", bufs=2) as rpool,
            tc.tile_pool(name="st_ps", bufs=2, space="PSUM") as stp,
            tc.tile_pool(name="oop_ps", bufs=4, space="PSUM") as opp,
        ):
            outT = outTp.tile([P, NQ_C, L], F32R)
            mask_sb = constsp.tile([P, 768], F32R)
            nc.sync.dma_start(mask_sb[:], maskA[:, :])
            ones_f32 = constsp.tile([1, P], F32)
            nc.vector.memset(ones_f32[:], 1.0)

            def attn_head(h):
                kvh = h // 2
                for p in range(NPAIR):
                    kbs = [0, 1] if p == 0 else [2 * p - 1, 2 * p, 2 * p + 1]
                    r0 = 1 if p == 0 else 0
                    lo = r0 * 256
                    qs = slice(p * 256, (p + 1) * 256)
                    st = stp.tile([P, 1024], F32, tag="st")
                    for i, kb in enumerate(kbs):
                        reg = r0 + i
                        nc.tensor.matmul(
                            st[:, reg * 256 : (reg + 1) * 256],
                            qkT[:, 4 + kvh, kb * P : (kb + 1) * P],
                            qkT[:, h, qs],
                            start=True,
                            stop=True,
                        )
                    a = apool.tile([P, 768], F32R, tag="a")
                    nc.scalar.activation(
                        a[:, lo:768], st[:, lo:768], Exp, scale=SCALE
                    )
                    nc.vector.tensor_mul(
                        a[:, lo:768], a[:, lo:768], mask_sb[:, lo:768]
                    )
                    for i, kb in enumerate(kbs):  # Z into st region 3
                        reg = r0 + i
                        nc.tensor.matmul(
                            st[0:1, 768:1024],
                            mask_sb[:, ONES_COL],
                            a[:, reg * 256 : (reg + 1) * 256],
                            start=(i == 0),
                            stop=(i == len(kbs) - 1),
                        )
                    recip = rpool.tile([1, 256], F32, tag="recip")
                    nc.vector.reciprocal_approx_fast(
                        out=recip[:], in_=st[0:1, 768:1024]
                    )
                    # broadcast 1/Z across partitions into region 0, then
                    # stage to SBUF (DVE reads only one PSUM operand)
                    nc.tensor.matmul(
                        st[:, 0:256],
                        ones_f32[0:1, :],
                        recip[0:1, :],
                        start=True,
                        stop=True,
                    )
                    zb_sb = rpool.tile([P, 256], F32, tag="zb")
                    nc.scalar.activation(zb_sb[:], st[:, 0:256], Copy)
                    o_ps = opp.tile([P, 512], F32, tag="oop")
                    for i, kb in enumerate(kbs):
                        reg = r0 + i
                        nc.tensor.matmul(
                            o_ps[:, 0:256],
                            vN[:, kb, kvh * HD : (kvh + 1) * HD],
                            a[:, reg * 256 : (reg + 1) * 256],
                            start=(i == 0),
                            stop=(i == len(kbs) - 1),
                        )
                    nc.vector.tensor_mul(
                        outT[:, h, qs], o_ps[:, 0:256], zb_sb[:]
                    )

            rope(0)
            rope(1)
            attn_head(0)
            rope(2)
            attn_head(1)
            rope(3)
            attn_head(2)
            attn_head(3)

            # ---- out-projection, wo streamed in 128-col chunks ----
            with (
                tc.tile_pool(name="wo", bufs=3) as wop,
                tc.tile_pool(name="oev", bufs=4) as oev,
            ):
                for d in range(DIM // P):
                    woc = wop.tile([P, NQ_C, P], F32R, tag="woc")
                    nc.sync.dma_start(woc[:], woT_t[:, :, d * P : (d + 1) * P])
                    for lb in range(LB):
                        ps = opp.tile([P, 512], F32, tag="oop")
                        for h in range(NQ_C):
                            nc.tensor.matmul(
                                ps[:],
                                woc[:, h, :],
                                outT[:, h, lb * 512 : (lb + 1) * 512],
                                start=(h == 0),
                                stop=(h == NQ_C - 1),
                            )
                        ot = oev.tile([P, 512], F32, tag="oe")
                        nc.scalar.activation(ot[:], ps[:], Copy)
                        nc.sync.dma_start(
                            outp[d * P : (d + 1) * P, lb * 512 : (lb + 1) * 512],
                            ot[:],
                        )

    nc.compile()
    return nc


_NC_CACHE = {}


def _get_nc():
    if "nc" not in _NC_CACHE:
        _NC_CACHE["nc"] = build_nc()
    return _NC_CACHE["nc"]


def _build_masks():
    k = np.arange(P)[:, None]  # partition = key pos within block
    r = np.arange(P)[None, :]  # free = query pos within block
    ut = (r <= k).astype(np.float32)  # delta = -1
    lt = (r >= k).astype(np.float32)  # delta = 0
    z = np.zeros((P, P), np.float32)
    ma = np.concatenate([ut, z], axis=1)
    mb = np.concatenate([lt, ut], axis=1)
    mc = np.concatenate([z, lt], axis=1)
    return np.ascontiguousarray(np.concatenate([ma, mb, mc], axis=1))


def _shard_inputs(**inputs):
    x = np.asarray(inputs["x"], np.float32)
    cos = np.asarray(inputs["cos"], np.float32)
    sin = np.asarray(inputs["sin"], np.float32)
    wq = np.asarray(inputs["wq"], np.float32)
    wk = np.asarray(inputs["wk"], np.float32)
    wv = np.asarray(inputs["wv"], np.float32)
    wo = np.asarray(inputs["wo"], np.float32)

    cosT = np.ascontiguousarray(cos.T)
    sT = np.ascontiguousarray(sin.T).copy()
    sT[: HD // 2] = -sT[: HD // 2]
    maskA = _build_masks()

    in_maps = []
    for c in range(8):
        b, g = c // 4, c % 4
        xT = np.ascontiguousarray(x[b].T)
        wq_g = wq[g * 512 : (g + 1) * 512]  # 4 q heads
        wk_g = wk[g * 256 : (g + 1) * 256]  # 2 kv heads
        wv_g = wv[g * 256 : (g + 1) * 256]
        wqkvT = np.ascontiguousarray(
            np.concatenate([wq_g, wk_g, wv_g], axis=0).T
        )
        woT = np.ascontiguousarray(wo[:, g * 512 : (g + 1) * 512].T)
        in_maps.append(
            {
                "xT": xT,
                "wqkvT": wqkvT,
                "woT": woT,
                "cosT": cosT,
                "sT": sT,
                "maskA": maskA,
            }
        )
    return in_maps


def kernel(**inputs):
    in_maps = _shard_inputs(**inputs)
    nc = _get_nc()
    res = run_bass_kernel_spmd(nc, in_maps, list(range(8)))
    outs = [r["outp"] for r in res.results]
    out = np.empty((B, L, DIM), np.float32)
    for b in range(B):
        acc = outs[b * 4].astype(np.float64)
        for g in range(1, 4):
            acc += outs[b * 4 + g]
        out[b] = acc.T.astype(np.float32)
    return out


# revision 18
# speedup vs baseline: 1.3983x; 1.0135x over previous
"""Trainium2 Bass kernel for sliding-window GQA attention block.

Problem (hardcoded):
  B=2, L=2048, DIM=2048, NH=16, NKV=8, HD=128, WIN=128
  out = ( softmax(mask(RoPE(xWq) @ RoPE(xWk)^T * hd^-0.5)) @ (xWv) ) @ Wo^T

Sharding: 8 cores = 2 batches x 4 head-groups (4 q heads + 2 kv heads each).
Each core computes a partial (over its head group) of out[b] in transposed
layout; host sums the 4 partials per batch and transposes back.

All device matmuls run in float32r (full rate at free-dim>=256, fp32 PSUM
accumulation). No on-device transposes:
  - x is passed as xT (DIM, L), weights pre-transposed host-side.
  - q, k are produced as qT/kT [hd, L]; V in natural [l, hd] layout via a
    second projection pass with xT as the stationary operand.
  - scores are computed transposed ST[k, q]; softmax sum over k is a PE
    matmul with a ones column into region 3 of the score PSUM tile; 1/Z is
    partition-broadcast with a K=1 PE outer product into region 0.

Scheduling (keeps TensorE dense so the HAM clock stays at 2.4 GHz):
  - weights and the first x chunk are per-k-tile tiles, matmuls k-outer, so
    the PE ramps with DMA arrival instead of waiting for whole-tile loads;
  - RoPE runs on 512-wide chunks fused right after each projection evict;
  - attention walks pairs descending so it overlaps the q-projection tail
    (which runs lb descending and reuses the kv-pass x chunks);
  - out-projection is interleaved per finished 512-wide l-chunk as dense
    PE filler between attention pairs.
"""

import sys

sys.path.insert(0, "/opt/trn_rl_repo")

import numpy as np

import concourse.bass as bass
import concourse.mybir as mybir
import concourse.tile as tile
from concourse import bacc
from concourse.bass_utils import run_bass_kernel_spmd

B, L, DIM = 2, 2048, 2048
NH, NKV, HD, WIN = 16, 8, 128, 128
P = 128
NQ_C = 4  # q heads per core
NKV_C = 2  # kv heads per core
KO = DIM // P  # 16 contraction tiles
LB = L // 512  # 4 l-chunks of 512
NPAIR = L // 256  # 8 query-block pairs
SCALE = float(HD) ** -0.5

F32 = mybir.dt.float32
F32R = mybir.dt.float32r

Copy = mybir.ActivationFunctionType.Copy
Exp = mybir.ActivationFunctionType.Exp

ONES_COL = slice(383, 384)  # all-ones column of Mb (LT col 127)


def build_nc():
    nc = bacc.Bacc(None, target_bir_lowering=False, debug=False)

    xT = nc.dram_tensor("xT", [DIM, L], F32R, kind="ExternalInput")
    wqkvT = nc.dram_tensor("wqkvT", [DIM, 1024], F32R, kind="ExternalInput")
    woT = nc.dram_tensor("woT", [NQ_C * HD, DIM], F32R, kind="ExternalInput")
    cosT = nc.dram_tensor("cosT", [HD, L], F32R, kind="ExternalInput")
    sT = nc.dram_tensor("sT", [HD, L], F32R, kind="ExternalInput")
    maskA = nc.dram_tensor("maskA", [P, 768], F32R, kind="ExternalInput")
    outp = nc.dram_tensor("outp", [DIM, L], F32, kind="ExternalOutput")

    xT_t = xT.rearrange("(ko p) l -> p ko l", p=P)
    wqkvT_t = wqkvT.rearrange("(ko p) m -> p ko m", p=P)
    woT_t = woT.rearrange("(ho p) d -> p ho d", p=P)

    with (
        tile.TileContext(nc) as tc,
        tc.tile_pool(name="persist", bufs=1) as persist,
        tc.tile_pool(name="trig", bufs=1) as trig,
        tc.tile_pool(name="rtmp", bufs=3) as rtmp,
    ):
        # qkT: douts 0..3 = qT heads, 4..5 = kT kv-heads; [hd, L] each
        qkT = persist.tile([P, 6, L], F32R)
        # vN: natural v [l(128), lblock(16), hd of 2 kv heads(256)]
        vN = persist.tile([P, KO, NKV_C * HD], F32R)

        cos_sb = trig.tile([P, L], F32R)
        s_sb = trig.tile([P, L], F32R)
        H = HD // 2

        def rope_chunk(d, lb):
            # in-place on a 512-wide chunk: base = base*cos + swap(base)*sT
            sl = slice(lb * 512, (lb + 1) * 512)
            base = qkT[:, d, sl]
            sw = rtmp.tile([P, 512], F32R, tag="sw")
            nc.sync.dma_start(sw[0:H, :], qkT[H:P, d, sl])
            nc.sync.dma_start(sw[H:P, :], qkT[0:H, d, sl])
            nc.vector.tensor_mul(base, base, cos_sb[:, sl])
            nc.vector.tensor_mul(sw[:], sw[:], s_sb[:, sl])
            nc.vector.tensor_add(base, base, sw[:])

        with tc.tile_pool(name="xcp", bufs=2) as xpool:
            xcs = {}

            # ---- Phase 1a: k projection (transposed) + v (natural) ----
            with (
                tc.tile_pool(name="wkv", bufs=1) as wpool,
                tc.tile_pool(name="pjkv", bufs=4, space="PSUM") as pjp,
            ):
                # first x chunk ahead of per-k weight tiles so the k-outer
                # matmul loop can start as soon as xc0 + wk0 land
                xc0 = xpool.tile([P, KO, 512], F32R, tag="xc", name="xc0")
                nc.sync.dma_start(xc0[:], xT_t[:, :, 0:512])
                xcs[0] = xc0
                wks = []
                for k in range(KO):
                    wk = wpool.tile([P, 512], F32R, tag=f"wk{k}", name=f"wk{k}")
                    nc.sync.dma_start(wk[:], wqkvT_t[:, k, 512:1024])
                    wks.append(wk)
                nc.sync.dma_start(cos_sb[:], cosT[:, :])
                nc.sync.dma_start(s_sb[:], sT[:, :])

                for lb in range(LB):
                    if lb == 0:
                        xc = xcs[0]
                    else:
                        xc = xpool.tile([P, KO, 512], F32R, tag="xc")
                        nc.sync.dma_start(
                            xc[:], xT_t[:, :, lb * 512 : (lb + 1) * 512]
                        )
                        xcs[lb] = xc
                    xck = [xc[:, k, :] for k in range(KO)]
                    kps = [pjp.tile([P, 512], F32, tag="pj", name=f"kp{lb}_{i}") for i in range(2)]
                    vps = [pjp.tile([P, 256], F32, tag="pjv", name=f"vp{lb}_{i}") for i in range(4)]
                    for k in range(KO):
                        st0, sp0 = (k == 0), (k == KO - 1)
                        for d in range(2):
                            nc.tensor.matmul(
                                kps[d][:],
                                wks[k][:, d * P : (d + 1) * P],
                                xck[k],
                                start=st0,
                                stop=sp0,
                            )
                        for j in range(4):
                            nc.tensor.matmul(
                                vps[j][:],
                                xck[k][:, j * P : (j + 1) * P],
                                wks[k][:, 256:512],
                                start=st0,
                                stop=sp0,
                            )
                    for d in range(2):
                        nc.scalar.activation(
                            qkT[:, 4 + d, lb * 512 : (lb + 1) * 512],
                            kps[d][:],
                            Copy,
                        )
                    for j in range(4):
                        nc.scalar.activation(vN[:, lb * 4 + j, :], vps[j][:], Copy)
                    for d in range(2):
                        rope_chunk(4 + d, lb)

            # ---- Phase 1b: q projection (reuses resident x chunks) ----
            with (
                tc.tile_pool(name="wq", bufs=1) as wpool,
                tc.tile_pool(name="pjq", bufs=8, space="PSUM") as pjp,
            ):
                wqs = []
                for k in range(KO):
                    wk = wpool.tile([P, 512], F32R, tag=f"wq{k}", name=f"wq{k}")
                    nc.sync.dma_start(wk[:], wqkvT_t[:, k, 0:512])
                    wqs.append(wk)
                for lb in (3, 2, 1, 0):
                    if lb >= 2:
                        xc = xcs[lb]
                    else:
                        xc = xpool.tile([P, KO, 512], F32R, tag="xc")
                        nc.sync.dma_start(
                            xc[:], xT_t[:, :, lb * 512 : (lb + 1) * 512]
                        )
                    qps = [pjp.tile([P, 512], F32, tag="pj", name=f"qp{lb}_{i}") for i in range(4)]
                    for k in range(KO):
                        for d in range(4):
                            nc.tensor.matmul(
                                qps[d][:],
                                wqs[k][:, d * P : (d + 1) * P],
                                xc[:, k, :],
                                start=(k == 0),
                                stop=(k == KO - 1),
                            )
                    for d in range(4):
                        nc.scalar.activation(
                            qkT[:, d, lb * 512 : (lb + 1) * 512], qps[d][:], Copy
                        )
                    for d in range(4):
                        rope_chunk(d, lb)

        # ---- attention (pairs descending to chase the q-projection) ----
        with (
            tc.tile_pool(name="outTp", bufs=1) as outTp,
            tc.tile_pool(name="wo", bufs=1) as wop,
            tc.tile_pool(name="consts", bufs=1) as constsp,
            tc.tile_pool(name="apool", bufs=4) as apool,
            tc.tile_pool(name="rpool", bufs=3) as rpool,
            tc.tile_pool(name="oev", bufs=4) as oev,
            tc.tile_pool(name="st_ps", bufs=2, space="PSUM") as stp,
            tc.tile_pool(name="oop_ps", bufs=4, space="PSUM") as opp,
        ):
            outT = outTp.tile([P, NQ_C, L], F32R)
            wo_sb = wop.tile([P, NQ_C, DIM], F32R)
            nc.sync.dma_start(wo_sb[:], woT_t[:, :, :])
            mask_sb = constsp.tile([P, 768], F32R)
            nc.sync.dma_start(mask_sb[:], maskA[:, :])
            ones_f32 = constsp.tile([1, P], F32)
            nc.vector.memset(ones_f32[:], 1.0)

            def attn_pair(p):
                kbs = [0, 1] if p == 0 else [2 * p - 1, 2 * p, 2 * p + 1]
                r0 = 1 if p == 0 else 0
                lo = r0 * 256
                qs = slice(p * 256, (p + 1) * 256)
                for h in range(NQ_C):
                    kvh = h // 2
                    st = stp.tile([P, 1024], F32, tag="st")
                    for i, kb in enumerate(kbs):
                        reg = r0 + i
                        nc.tensor.matmul(
                            st[:, reg * 256 : (reg + 1) * 256],
                            qkT[:, 4 + kvh, kb * P : (kb + 1) * P],
                            qkT[:, h, qs],
                            start=True,
                            stop=True,
                        )
                    a = apool.tile([P, 768], F32R, tag="a")
                    nc.scalar.activation(
                        a[:, lo:768], st[:, lo:768], Exp, scale=SCALE
                    )
                    nc.vector.tensor_mul(
                        a[:, lo:768], a[:, lo:768], mask_sb[:, lo:768]
                    )
                    for i, kb in enumerate(kbs):  # Z into st region 3
                        reg = r0 + i
                        nc.tensor.matmul(
                            st[0:1, 768:1024],
                            mask_sb[:, ONES_COL],
                            a[:, reg * 256 : (reg + 1) * 256],
                            start=(i == 0),
                            stop=(i == len(kbs) - 1),
                        )
                    recip = rpool.tile([1, 256], F32, tag="recip")
                    nc.vector.reciprocal_approx_fast(
                        out=recip[:], in_=st[0:1, 768:1024]
                    )
                    nc.tensor.matmul(  # 1/Z partition-broadcast to region 0
                        st[:, 0:256],
                        ones_f32[0:1, :],
                        recip[0:1, :],
                        start=True,
                        stop=True,
                    )
                    zb_sb = rpool.tile([P, 256], F32, tag="zb")
                    nc.scalar.activation(zb_sb[:], st[:, 0:256], Copy)
                    o_ps = opp.tile([P, 512], F32, tag="oop")
                    for i, kb in enumerate(kbs):
                        reg = r0 + i
                        nc.tensor.matmul(
                            o_ps[:, 0:256],
                            vN[:, kb, kvh * HD : (kvh + 1) * HD],
                            a[:, reg * 256 : (reg + 1) * 256],
                            start=(i == 0),
                            stop=(i == len(kbs) - 1),
                        )
                    nc.vector.tensor_mul(
                        outT[:, h, qs], o_ps[:, 0:256], zb_sb[:]
                    )

            def outproj(lb):
                for d in range(DIM // P):
                    ps = opp.tile([P, 512], F32, tag="oop")
                    for h in range(NQ_C):
                        nc.tensor.matmul(
                            ps[:],
                            wo_sb[:, h, d * P : (d + 1) * P],
                            outT[:, h, lb * 512 : (lb + 1) * 512],
                            start=(h == 0),
                            stop=(h == NQ_C - 1),
                        )
                    ot = oev.tile([P, 512], F32, tag="oe")
                    nc.scalar.activation(ot[:], ps[:], Copy)
                    nc.sync.dma_start(
                        outp[d * P : (d + 1) * P, lb * 512 : (lb + 1) * 512],
                        ot[:],
                    )

            for p in range(NPAIR - 1, -1, -1):
                attn_pair(p)
                if p % 2 == 0:
                    outproj(p // 2)

    nc.compile()
    return nc


_NC_CACHE = {}


def _get_nc():
    if "nc" not in _NC_CACHE:
        _NC_CACHE["nc"] = build_nc()
    return _NC_CACHE["nc"]


def _build_masks():
    k = np.arange(P)[:, None]  # partition = key pos within block
    r = np.arange(P)[None, :]  # free = query pos within block
    ut = (r <= k).astype(np.float32)  # delta = -1
    lt = (r >= k).astype(np.float32)  # delta = 0
    z = np.zeros((P, P), np.float32)
    ma = np.concatenate([ut, z], axis=1)
    mb = np.concatenate([lt, ut], axis=1)
    mc = np.concatenate([z, lt], axis=1)
    return np.ascontiguousarray(np.concatenate([ma, mb, mc], axis=1))


def _shard_inputs(**inputs):
    x = np.asarray(inputs["x"], np.float32)
    cos = np.asarray(inputs["cos"], np.float32)
    sin = np.asarray(inputs["sin"], np.float32)
    wq = np.asarray(inputs["wq"], np.float32)
    wk = np.asarray(inputs["wk"], np.float32)
    wv = np.asarray(inputs["wv"], np.float32)
    wo = np.asarray(inputs["wo"], np.float32)

    cosT = np.ascontiguousarray(cos.T)
    sT = np.ascontiguousarray(sin.T).copy()
    sT[: HD // 2] = -sT[: HD // 2]
    maskA = _build_masks()

    in_maps = []
    for c in range(8):
        b, g = c // 4, c % 4
        xT = np.ascontiguousarray(x[b].T)
        wq_g = wq[g * 512 : (g + 1) * 512]  # 4 q heads
        wk_g = wk[g * 256 : (g + 1) * 256]  # 2 kv heads
        wv_g = wv[g * 256 : (g + 1) * 256]
        wqkvT = np.ascontiguousarray(
            np.concatenate([wq_g, wk_g, wv_g], axis=0).T
        )
        woT = np.ascontiguousarray(wo[:, g * 512 : (g + 1) * 512].T)
        in_maps.append(
            {
                "xT": xT,
                "wqkvT": wqkvT,
                "woT": woT,
                "cosT": cosT,
                "sT": sT,
                "maskA": maskA,
            }
        )
    return in_maps


def kernel(**inputs):
    in_maps = _shard_inputs(**inputs)
    nc = _get_nc()
    res = run_bass_kernel_spmd(nc, in_maps, list(range(8)))
    outs = [r["outp"] for r in res.results]
    out = np.empty((B, L, DIM), np.float32)
    for b in range(B):
        acc = outs[b * 4].astype(np.float64)
        for g in range(1, 4):
            acc += outs[b * 4 + g]
        out[b] = acc.T.astype(np.float32)
    return out
